# revision 31
# baseline (speedup 1.0000x reference)
"""Trainium2 Bass kernel for CrossAttention (B=2, T=S=2048, E=1024, H=16, D=64).

Sharding: 8 cores = 2 (batch) x 4 (head groups of 4 heads).
Each core computes, for its (b, g):
  - Q/K projections in feature-major layout: QT/KT = [256, 2048]
  - V projection in sequence-major layout with an appended ones column per
    head (gives the softmax denominator for free from the attn@V matmul)
  - block-causal flash-style attention with exact per-block causal widths:
    scores / exp / attn@V only touch the valid column range of diagonal
    blocks; the 128-wide staircase strip is masked with a [128,128]
    triangular multiply on VectorE after the exp
  - output projection partial: [1024, 2048] f16, summed on host

Scores matmuls are K=64 so the two local head-pairs run concurrently on
(64,0)/(0,0) PE row-tiles.  All DMA layouts are host-prepped so every
descriptor moves contiguous >=1KB per-partition segments.
"""

import ml_dtypes
import numpy as np

import concourse.bass as bass
import concourse.bacc as bacc
import concourse.mybir as mybir
import concourse.tile as tile
from concourse.bass_utils import run_bass_kernel_spmd


P = 128
T = 2048          # target length
S = 2048          # source length
E = 1024          # embed dim
D = 64            # head dim
GC = 256          # channels per group (4 heads * 64)
NHL = 4           # heads per core (local)
KB = E // P       # 8 full k-blocks for the E contraction
TJ = 512          # t-chunk width
NTJ = T // TJ     # 4
NSB = S // P      # 16 s-blocks
VC = NHL * (D + 1)  # 260 = V-projection cols (64 V + 1 ones per head)
VW = 3 * D + 1  # 193 = stored V cols per head-pair [v0|ones0|zeros(63)|ones1|v1]
SCALE = float(D) ** -0.5  # 0.125

F32 = mybir.dt.float32
F32R = mybir.dt.float32r
F16 = mybir.dt.float16

DT = F16          # operand dtype everywhere on the PE


def _build_program():
    nc = bacc.Bacc()

    xq = nc.dram_tensor("xq_t", [P, KB, T], DT, kind="ExternalInput")
    xk = nc.dram_tensor("xk_t", [P, KB, S], DT, kind="ExternalInput")
    xv = nc.dram_tensor("xv_t", [P, KB, S], DT, kind="ExternalInput")
    wq = nc.dram_tensor("wq_t", [P, KB, GC], DT, kind="ExternalInput")
    wk = nc.dram_tensor("wk_t", [P, KB, GC], DT, kind="ExternalInput")
    wv = nc.dram_tensor("wv_t", [P, KB, VC], DT, kind="ExternalInput")
    wvl = nc.dram_tensor("wvl_t", [1, VC], DT, kind="ExternalInput")
    wo = nc.dram_tensor("wo_t", [P, 2, E], DT, kind="ExternalInput")
    # [P, P] upper-triangular keep mask (t >= s) for the staircase strip
    tri = nc.dram_tensor("tri", [P, P], DT, kind="ExternalInput")
    # broadcast weights: row 63 = [0]*64+[1]*64, row 64 = [1]*64+[0]*64
    bcl = nc.dram_tensor("bcl", [P, P], DT, kind="ExternalInput")
    # key-padding 0/1 columns per s-block plus per-channel q/k biases
    padb = nc.dram_tensor("padb", [P, NSB + 4], F32, kind="ExternalInput")
    out_t = nc.dram_tensor("out_t", [E, T], F16, kind="ExternalOutput")

    with tile.TileContext(nc) as tc:
        with (
            tc.tile_pool(name="consts", bufs=1) as cpool,
            tc.tile_pool(name="xs", bufs=1) as xpool,
            tc.tile_pool(name="persist", bufs=1) as ppool,
            tc.tile_pool(name="expw", bufs=6) as epool,
            tc.tile_pool(name="ao", bufs=1) as apool,
            tc.tile_pool(name="ft", bufs=4) as fpool,
            tc.tile_pool(name="ps", bufs=1, space="PSUM") as pspool,
        ):
            # ---- constants / weights to SBUF (all contiguous layouts) ----
            wq_sb = cpool.tile([P, KB, GC], DT, name="wq_sb")
            wk_sb = cpool.tile([P, KB, GC], DT, name="wk_sb")
            wv_sb = cpool.tile([P, KB + 1, VC], DT, name="wv_sb")
            wo_sb = cpool.tile([P, 2, E], DT, name="wo_sb")
            tri_sb = cpool.tile([P, P], DT, name="tri_sb")
            bcl_sb = cpool.tile([P, P], DT, name="bcl_sb")
            padb_sb = cpool.tile([P, NSB + 4], F32, name="padb_sb")
            ones_sb = cpool.tile([1, P], DT, name="ones_sb")

            # ---- persistent activations ----
            qt_sb = ppool.tile([P, 2, T], DT, name="qt_sb")
            kt_sb = ppool.tile([P, 2, S], DT, name="kt_sb")
            # per head-pair: [v0(64) | ones0 | zeros(63) | ones1 | v1(64)]
            v_sb = ppool.tile([P, NSB, 2, VW], DT, name="v_sb")
            aoTn = ppool.tile([P, 2, T], DT, name="aoTn")

            # ---- x chunk staging (ring per tensor) ----
            def fetch_x(x_dram, j, tag):
                t_ = xpool.tile([P, KB, TJ], DT, tag=tag, name=tag, bufs=2)
                nc.sync.dma_start(t_[:], x_dram[:, :, j * TJ : (j + 1) * TJ])
                return t_

            xq_t, xk_t, xv_t = {}, {}, {}

            def prefetch(j):
                if j < NTJ:
                    if j not in xk_t:
                        xk_t[j] = fetch_x(xk, j, "xk")
                    if j not in xv_t:
                        xv_t[j] = fetch_x(xv, j, "xv")
                    if j not in xq_t:
                        xq_t[j] = fetch_x(xq, j, "xq")

            # first-needed-first DMA order: xk0 + wk gate the first matmul
            xk_t[0] = fetch_x(xk, 0, "xk")
            nc.sync.dma_start(wk_sb[:], wk[:])
            xq_t[0] = fetch_x(xq, 0, "xq")
            nc.sync.dma_start(wq_sb[:], wq[:])
            xv_t[0] = fetch_x(xv, 0, "xv")
            nc.sync.dma_start(wv_sb[:, :KB, :], wv[:])
            nc.sync.dma_start(wv_sb[0:1, KB, :], wvl[:])
            nc.sync.dma_start(padb_sb[:], padb[:])
            nc.sync.dma_start(tri_sb[:], tri[:])
            nc.sync.dma_start(wo_sb[:], wo[:])
            nc.sync.dma_start(bcl_sb[:], bcl[:])
            nc.any.memset(ones_sb[:], 1.0)
            # zero the av-lhsT spacer columns once (never rewritten)
            nc.any.memset(v_sb[:, :, :, D + 2 : 2 * D + 2], 0.0)
            prefetch(1)

            def emit_qkv(j):
                jsl = slice(j * TJ, (j + 1) * TJ)
                # K projection (channel-major)
                for w_sb, x_t, dst, bcol in (
                    (wk_sb, xk_t[j], kt_sb, NSB + 2),
                    (wq_sb, xq_t[j], qt_sb, NSB),
                ):
                    for mc in range(2):
                        ps = pspool.tile([P, TJ], F32, tag="ps_pr", name="ps_pr", bufs=2)
                        for kb in range(KB):
                            nc.tensor.matmul(
                                ps[:],
                                lhsT=w_sb[:, kb, mc * P : (mc + 1) * P],
                                rhs=x_t[:, kb, :],
                                start=(kb == 0),
                                stop=(kb == KB - 1),
                            )
                        nc.vector.tensor_scalar_add(
                            dst[:, mc, jsl],
                            ps[:],
                            padb_sb[:, bcol + mc : bcol + mc + 1],
                        )
                # V projection (sequence-major; per hp: [v0|ones0|ones1|v1])
                for ii in range(TJ // P):
                    i = j * (TJ // P) + ii
                    ps = pspool.tile([P, 2, VC // 2], F32, tag="ps_pr", name="ps_v", bufs=2)
                    for kb in range(KB):
                        nc.tensor.matmul(
                            ps[:],
                            lhsT=xv_t[j][:, kb, ii * P : (ii + 1) * P],
                            rhs=wv_sb[:, kb, :],
                            start=(kb == 0),
                            stop=False,
                        )
                    nc.tensor.matmul(
                        ps[:],
                        lhsT=ones_sb[0:1, 0:P],
                        rhs=wv_sb[0:1, KB, :],
                        start=False,
                        stop=True,
                    )
                    # storage per hp: [v0(0:64)|ones0(64)|ones1(65)|zeros|v1(129:193)]
                    nc.vector.tensor_scalar_mul(
                        v_sb[:, i, :, 0 : D + 2],
                        ps[:, :, 0 : D + 2],
                        padb_sb[:, i : i + 1],
                    )
                    nc.vector.tensor_scalar_mul(
                        v_sb[:, i, :, 2 * D + 1 : VW],
                        ps[:, :, D + 2 : VC // 2],
                        padb_sb[:, i : i + 1],
                    )
                prefetch(j + 2)

            def emit_attention(hp, j):
                jsl = slice(j * TJ, (j + 1) * TJ)
                npairs = 2 * j + 2
                av_ps = [
                    pspool.tile([P, TJ], F32, tag=f"ps_av{lh}", name="ps_av", bufs=1)
                    for lh in range(2)
                ]
                # reciprocal rows live at partitions 63 (lh1) / 64 (lh0)
                rcp = apool.tile([D + 1, TJ], F32, tag="rcp", name="rcp", bufs=2)
                ets = {}

                def emit_scores_pair(m):
                    # pair-level causal offset: block u=0's strip position
                    off = P * max(2 * m - 4 * j, 0)
                    ps2s = [
                        pspool.tile([P, 2, TJ], F32, tag="ps_sc", name="ps_sc", bufs=2)
                        for _ in range(2)
                    ]
                    for u in range(2):
                        i = 2 * m + u
                        for lh in range(2):
                            base = D * lh
                            nc.tensor.matmul(
                                ps2s[lh][:, u, off:],
                                lhsT=kt_sb[base : base + D, hp, i * P : (i + 1) * P],
                                rhs=qt_sb[base : base + D, hp, j * TJ + off : (j + 1) * TJ],
                                start=True,
                                stop=True,
                                tile_position=(base, 0),
                            )
                    for lh in range(2):
                        et2 = epool.tile([P, 2, TJ], DT, tag="exp", name="et2")
                        nc.scalar.activation(
                            et2[:, :, off:],
                            ps2s[lh][:, :, off:],
                            mybir.ActivationFunctionType.Exp,
                            scale=SCALE,
                        )
                        for u in range(2):
                            r = 2 * m + u - 4 * j
                            if r >= 0:
                                # causal staircase strip: [128,128] tri mul
                                so = P * r
                                nc.vector.tensor_mul(
                                    out=et2[:, u, so : so + P],
                                    in0=et2[:, u, so : so + P],
                                    in1=tri_sb[:],
                                )
                        ets[m, lh] = et2

                def emit_av_pair(m, lh):
                    et2 = ets.pop((m, lh))
                    for u in range(2):
                        i = 2 * m + u
                        off = P * max(i - 4 * j, 0)
                        if lh == 0:
                            # lhsT [v0|ones0]: values -> parts 0-63, denom -> 64
                            nc.tensor.matmul(
                                av_ps[0][: D + 1, off:],
                                lhsT=v_sb[:, i, hp, 0 : D + 1],
                                rhs=et2[:, u, off:],
                                start=(i == 0),
                                stop=(i == 2 * npairs - 1),
                            )
                        else:
                            # lhsT [ones1|zeros|v1]: denom -> 0, values -> 64-127
                            nc.tensor.matmul(
                                av_ps[1][:, off:],
                                lhsT=v_sb[:, i, hp, D + 1 : VW],
                                rhs=et2[:, u, off:],
                                start=(i == 0),
                                stop=(i == 2 * npairs - 1),
                            )

                # software pipeline: attn@V trails scores/exp by two pairs
                for m in range(npairs):
                    emit_scores_pair(m)
                    if m >= 2:
                        for lh in range(2):
                            emit_av_pair(m - 2, lh)
                for m in range(max(npairs - 2, 0), npairs):
                    for lh in range(2):
                        emit_av_pair(m, lh)
                    if m == npairs - 1:
                        # denominators PSUM -> SBUF (in-lane), then reciprocal
                        den = apool.tile([D + 1, TJ], F32, tag="den", name="den", bufs=2)
                        nc.vector.tensor_copy(
                            out=den[D : D + 1, :], in_=av_ps[0][D : D + 1, :]
                        )
                        nc.vector.tensor_copy(out=den[0:1, :], in_=av_ps[1][0:1, :])
                        # lh0 denominator row partition 64 -> 1 (SBUF DMA hop)
                        nc.sync.dma_start(den[1:2, :], den[D : D + 1, :])
                        nc.vector.reciprocal_approx_fast(
                            rcp[0:2, :], den[0:2, :]
                        )

                # broadcast the reciprocal rows across partitions with one
                # K=2 f16 matmul at base partition 0:
                # parts 64-127 <- row 0 (lh1), parts 0-63 <- row 1 (lh0)
                rcp16 = apool.tile([2, TJ], DT, tag="rcp16", name="rcp16", bufs=2)
                nc.vector.tensor_copy(out=rcp16[:], in_=rcp[0:2, :])
                rb_ps = pspool.tile([P, TJ], F32, tag="ps_pr", name="rb_ps", bufs=2)
                nc.tensor.matmul(
                    rb_ps[:],
                    lhsT=bcl_sb[0:2, :],
                    rhs=rcp16[:],
                    start=True,
                    stop=True,
                )
                # evacuate values to SBUF (in-lane), then one full-width
                # normalize mul against the PSUM broadcast bank
                aoS = apool.tile([P, TJ], F32, tag="aoS", name="aoS", bufs=2)
                nc.vector.tensor_copy(out=aoS[0:D, :], in_=av_ps[0][0:D, :])
                nc.vector.tensor_copy(
                    out=aoS[D : 2 * D, :], in_=av_ps[1][D : 2 * D, :]
                )
                nc.vector.tensor_mul(
                    out=aoTn[:, hp, jsl],
                    in0=aoS[:],
                    in1=rb_ps[:],
                )

            def emit_oproj(j):
                jsl = slice(j * TJ, (j + 1) * TJ)
                for mc in range(KB):
                    ps = pspool.tile([P, TJ], F32, tag="ps_pr", name="ps_o", bufs=2)
                    for cc in range(2):
                        nc.tensor.matmul(
                            ps[:],
                            lhsT=wo_sb[:, cc, mc * P : (mc + 1) * P],
                            rhs=aoTn[:, cc, jsl],
                            start=(cc == 0),
                            stop=(cc == 1),
                        )
                    oc = fpool.tile([P, TJ], F16, tag="oc", name="oc", bufs=4)
                    if mc % 2 == 0:
                        nc.vector.tensor_copy(out=oc[:], in_=ps[:])
                    else:
                        nc.scalar.copy(oc[:], ps[:])
                    nc.sync.dma_start(
                        out_t[mc * P : (mc + 1) * P, jsl], oc[:]
                    )

            # ---- phase-interleaved emission ----
            emit_qkv(0)
            emit_qkv(1)
            emit_attention(0, 0)
            emit_attention(1, 0)
            emit_qkv(2)
            emit_oproj(0)
            emit_attention(0, 1)
            emit_attention(1, 1)
            emit_qkv(3)
            emit_oproj(1)
            emit_attention(0, 2)
            emit_attention(1, 2)
            emit_attention(0, 3)
            emit_oproj(2)
            emit_attention(1, 3)
            emit_oproj(3)

    nc.compile()
    return nc


_NC_CACHE = None


def _get_nc():
    global _NC_CACHE
    if _NC_CACHE is None:
        _NC_CACHE = _build_program()
    return _NC_CACHE


def _pack_pkt(a):
    """[R*P, C] -> [P, R, C] partition-major (row r*P+p -> (p, r))."""
    r = a.shape[0] // P
    return np.ascontiguousarray(a[: r * P].reshape(r, P, -1).transpose(1, 0, 2))


def _make_in_maps(query, key, value, key_padding_mask, Wq, bq, Wk, bk, Wv, bv, Wo, bo):
    f32, f16 = np.float32, np.float16
    query = np.asarray(query, f32)
    key = np.asarray(key, f32)
    value = np.asarray(value, f32)
    kpm = np.asarray(key_padding_mask, bool)
    Wq, bq = np.asarray(Wq, f32), np.asarray(bq, f32)
    Wk, bk = np.asarray(Wk, f32), np.asarray(bk, f32)
    Wv, bv = np.asarray(Wv, f32), np.asarray(bv, f32)
    Wo = np.asarray(Wo, f32)

    tri_np = (np.arange(P)[None, :] >= np.arange(P)[:, None]).astype(f16)
    bcl_np = np.zeros((P, P), f16)
    bcl_np[0, D:] = 1.0       # row 0 (lh1 rcp) -> partitions 64-127
    bcl_np[1, :D] = 1.0       # row 1 (lh0 rcp) -> partitions 0-63

    xq_b = [_pack_pkt(query[b].T.astype(f16)) for b in range(2)]
    xk_b = [_pack_pkt(key[b].T.astype(f16)) for b in range(2)]
    xv_b = [_pack_pkt(value[b].T.astype(f16)) for b in range(2)]

    in_maps = []
    for c in range(8):
        b, g = divmod(c, 4)
        cols = slice(g * GC, (g + 1) * GC)

        wq_t = _pack_pkt(Wq[cols, :].T.astype(f16))
        wk_t = _pack_pkt(Wk[cols, :].T.astype(f16))

        # per head-pair hp the 130 psum cols are [v0(64)|ones0|ones1|v1(64)]
        wv_full = np.zeros((E + 1, VC), f16)
        for hp2 in range(2):
            base = hp2 * (VC // 2)
            ch0 = slice(g * GC + (2 * hp2) * D, g * GC + (2 * hp2 + 1) * D)
            ch1 = slice(g * GC + (2 * hp2 + 1) * D, g * GC + (2 * hp2 + 2) * D)
            wv_full[:E, base : base + D] = Wv[ch0, :].T
            wv_full[E, base : base + D] = bv[ch0]
            wv_full[E, base + D] = 1.0       # ones0 -> denom of head 2hp
            wv_full[E, base + D + 1] = 1.0   # ones1 -> denom of head 2hp+1
            wv_full[:E, base + D + 2 : base + 2 * D + 2] = Wv[ch1, :].T
            wv_full[E, base + D + 2 : base + 2 * D + 2] = bv[ch1]
        wv_t = _pack_pkt(wv_full[:E])
        wvl_t = np.ascontiguousarray(wv_full[E : E + 1])

        wo_t = _pack_pkt(Wo[:, cols].T.astype(f16))

        padb_np = np.where(kpm[b], 0.0, 1.0).astype(f32).reshape(NSB, P).T
        biases = np.stack(
            [bq[cols][:P], bq[cols][P:], bk[cols][:P], bk[cols][P:]], axis=1
        ).astype(f32)
        padb_np = np.ascontiguousarray(np.concatenate([padb_np, biases], axis=1))

        in_maps.append(
            {
                "xq_t": xq_b[b],
                "xk_t": xk_b[b],
                "xv_t": xv_b[b],
                "wq_t": wq_t,
                "wk_t": wk_t,
                "wv_t": wv_t,
                "wvl_t": wvl_t,
                "wo_t": wo_t,
                "tri": tri_np,
                "bcl": bcl_np,
                "padb": padb_np,
            }
        )
    return in_maps


def kernel(**inputs) -> np.ndarray:
    nc = _get_nc()
    in_maps = _make_in_maps(**inputs)
    res = run_bass_kernel_spmd(nc, in_maps, core_ids=list(range(8)))
    bo = np.asarray(inputs["bo"], np.float32)
    B = inputs["query"].shape[0]
    out = np.zeros((B, T, E), np.float32)
    for c in range(8):
        b = c // 4
        out[b] += res.results[c]["out_t"].astype(np.float32).T
    out += bo[None, None, :]
    return out
